# revision 3
# baseline (speedup 1.0000x reference)
"""Causal self-attention (B=2, T=2048, D=2048, H=16, HD=128) on 8 TRN2 cores.

Tensor-parallel over heads (2 heads/core), token-parallel QKV projection,
with the collectives merged into exactly two AllToAlls (one 6MB qkv+v,
one 2MB y) - collective dispatch/serialization is a dominant per-iteration
cost in this runtime.

Engine balance: every ACT op (Square / Ln / Exp / none else) lives in the
single `natural_log_exp_and_others` table set - rsqrt for RMS-norm is
computed as exp(-0.5*ln(ms+eps)), so the strict-FIFO scalar engine never
pays the ~2.7us ACT table reload, regardless of scheduling order.
PSUM->SBUF copies run on the vector engine (ACT is ~9x slower at copies).
DMAs are batched via rearranged access patterns (v/raw/yt reads one DMA
per A2A slot, output one DMA per row-block).

  - x ships as a per-core 512-token slice of xT (2MB bf16),
  - weights/cos/sin/mask are baked into the NEFF as Const tensors,
  - output returns bf16 (cast to f32 on host).
Matmul layouts: contractions on the partition dim; scores transposed
(S^T = k-chunk @ q^T); softmax denominator via all-ones matmul;
normalization folds into the PSUM->SBUF copy. Matmuls bf16, stats fp32.
"""

import numpy as np

B, T, D = 2, 2048, 2048
H, HD = 16, 128
N_CORES = 8
HPC = H // N_CORES          # heads per core
NT = B * T                  # 4096 tokens, b-major
TS = NT // N_CORES          # 512-token slice per core
DC = D // 128               # 16 contraction chunks
NTT = NT // 512             # 8 token tiles
KT_PER_B = T // 128         # 16 k-tiles per batch row

_CACHE = {}


def _build(scale: float, woT, wqkT, wvT, cs_full, m0, reps: int = 1):
    import concourse.bacc as bacc
    import concourse.mybir as mybir
    import concourse.tile as tile

    f32 = mybir.dt.float32
    MM = mybir.dt.bfloat16
    EPS = float(np.finfo(np.float32).eps)

    nc = bacc.Bacc("TRN2", target_bir_lowering=False, debug=False,
                   num_devices=N_CORES)

    xs_d = nc.dram_tensor("xs", [D, TS], MM, kind="ExternalInput")
    y_d = nc.dram_tensor("y", [TS, D], MM, kind="ExternalOutput")
    # model-load-time constants: no per-execution staging cost
    wo_d = nc.inline_tensor(woT, name="woc")          # [D, D] = Wo^T
    wqk_c = nc.inline_tensor(wqkT, name="wqkc")       # [D, 2D] = (W0||W1)^T
    wv_c = nc.inline_tensor(wvT, name="wvc")          # [D, D] = W2^T
    cs_c = nc.inline_tensor(cs_full, name="csc")      # [256, NT]
    m0_c = nc.inline_tensor(m0, name="m0c")           # [128, 512] triangle

    Sq = mybir.ActivationFunctionType.Square
    Ln = mybir.ActivationFunctionType.Ln
    Exp = mybir.ActivationFunctionType.Exp

    rg = [list(range(N_CORES))]

    with tile.TileContext(nc) as tc:
        with tc.tile_pool(name="dram", bufs=1, space="DRAM") as dram, \
             tc.tile_pool(name="res", bufs=1) as res:
            # merged qkv A2A: slot s rows 0-511 = q/k dims of core s
            # (q h0|q h1|k h0|k h1), cols = my 512 tokens; rows 512-767 =
            # v token-major quadrants [128 tok x 256 vdim] x (2x2)
            aqv_i = dram.tile([N_CORES, 768, 512], MM, tag="aqvi", name="aqvi")
            aqv_o = dram.tile([N_CORES, 768, 512], MM, tag="aqvo", name="aqvo")
            # merged y A2A: slot s = [my 2 heads x 128 rows, tokens of s]
            ay_i = dram.tile([N_CORES, HPC * HD, TS], MM, tag="ayi", name="ayi")
            ay_o = dram.tile([N_CORES, HPC * HD, TS], MM, tag="ayo", name="ayo")

            qk_sb = res.tile([128, 4 * NT], MM, tag="qk")
            v_sb = res.tile([128, (NT // 128) * (HPC * HD)], MM, tag="v")
            m0_sb = res.tile([128, 512], MM, tag="m0")
            ones_sb = res.tile([128, 128], MM, tag="ones")
            eps_sb = res.tile([128, 1], f32, tag="eps")
            nc.vector.memset(eps_sb[:], EPS)
            nc.vector.memset(ones_sb[:], 1.0)
            nc.sync.dma_start(out=m0_sb[:], in_=m0_c[:, :])

            for _rep in range(reps):
                # ------- Phase 1: token-parallel QKV into one A2A -------
                with tc.tile_pool(name="p1", bufs=1) as p1, \
                     tc.tile_pool(name="wp", bufs=2) as wp, \
                     tc.tile_pool(name="cp", bufs=4) as cp, \
                     tc.tile_pool(name="ps1", bufs=3, space="PSUM") as ps1:
                    xsb = p1.tile([128, DC * 512], MM, tag="xsb")
                    nc.sync.dma_start(
                        out=xsb[:].rearrange("p (c f) -> p c f", f=512),
                        in_=xs_d[:, :].rearrange("(c p) f -> p c f", p=128))

                    # v = x @ Wv -> [my tok, vdim]; dest 2ob gets cols 0:256,
                    # dest 2ob+1 cols 256:512; quadrant (t4//2, t4%2)
                    for ob in range(4):
                        wvb = wp.tile([128, DC * 512], MM, tag="wb", name="wvb")
                        nc.sync.dma_start(
                            out=wvb[:].rearrange("p (c f) -> p c f", f=512),
                            in_=wv_c[:, ob * 512:(ob + 1) * 512]
                                .rearrange("(c p) f -> p c f", p=128))
                        for t4 in range(4):
                            vps = ps1.tile([128, 512], f32, tag="vps")
                            for dc in range(DC):
                                nc.tensor.matmul(
                                    vps[:],
                                    xsb[:, dc * 512 + t4 * 128: dc * 512 + (t4 + 1) * 128],
                                    wvb[:, dc * 512:(dc + 1) * 512],
                                    start=(dc == 0), stop=(dc == DC - 1))
                            vb = cp.tile([128, 512], MM, tag="vb")
                            nc.vector.tensor_copy(vb[:], vps[:])
                            r0 = 512 + (t4 // 2) * 128
                            c0 = (t4 % 2) * 256
                            nc.sync.dma_start(
                                out=aqv_i[2 * ob, r0:r0 + 128, c0:c0 + 256],
                                in_=vb[:, 0:256])
                            nc.sync.dma_start(
                                out=aqv_i[2 * ob + 1, r0:r0 + 128, c0:c0 + 256],
                                in_=vb[:, 256:512])

                    # q||k = x @ (W0||W1): [out-dim 128, my 512 tok] chunks
                    for ob in range(8):
                        wqb = wp.tile([128, DC * 512], MM, tag="wb", name="wqb")
                        nc.sync.dma_start(
                            out=wqb[:].rearrange("p (c f) -> p c f", f=512),
                            in_=wqk_c[:, ob * 512:(ob + 1) * 512]
                                .rearrange("(c p) f -> p c f", p=128))
                        for oc in range(4):
                            qps = ps1.tile([128, 512], f32, tag="qps")
                            for dc in range(DC):
                                nc.tensor.matmul(
                                    qps[:],
                                    wqb[:, dc * 512 + oc * 128: dc * 512 + (oc + 1) * 128],
                                    xsb[:, dc * 512:(dc + 1) * 512],
                                    start=(dc == 0), stop=(dc == DC - 1))
                            qb = cp.tile([128, 512], MM, tag="qb")
                            nc.vector.tensor_copy(qb[:], qps[:])
                            o = ob * 4 + oc               # global 128-out chunk
                            if o < 16:
                                s, row = o // 2, (o % 2) * 128
                            else:
                                s, row = (o - 16) // 2, 256 + ((o - 16) % 2) * 128
                            nc.sync.dma_start(
                                out=aqv_i[s, row:row + 128, :], in_=qb[:])
                    nc.gpsimd.collective_compute(
                        "AllToAll", mybir.AluOpType.bypass,
                        replica_groups=rg,
                        ins=[aqv_i.opt()], outs=[aqv_o.opt()])

                # ----- Phase 2: norm+rotary, attention, y A2A, o-proj -----
                with tc.tile_pool(name="st", bufs=3) as st, \
                     tc.tile_pool(name="p2", bufs=4) as p2, \
                     tc.tile_pool(name="p2b", bufs=2) as p2b, \
                     tc.tile_pool(name="pss", bufs=2, space="PSUM") as pss, \
                     tc.tile_pool(name="psd", bufs=2, space="PSUM") as psd, \
                     tc.tile_pool(name="psy", bufs=2, space="PSUM") as psy, \
                     tc.tile_pool(name="p3", bufs=1) as p3, \
                     tc.tile_pool(name="wop", bufs=4) as wop, \
                     tc.tile_pool(name="ob", bufs=2) as obp, \
                     tc.tile_pool(name="ps3", bufs=2, space="PSUM") as ps3:
                    cs_sb = p3.tile([128, 2 * NT], MM, tag="cs")
                    nc.sync.dma_start(
                        out=cs_sb[:].rearrange("p (c f) -> p c f", f=NT),
                        in_=cs_c[:, :].rearrange("(c p) f -> p c f", p=128))
                    # v: one DMA per slot; quadrant layout maps exactly onto
                    # v_sb's (tcg*256) column order
                    for g in range(NTT):
                        nc.sync.dma_start(
                            out=v_sb[:, g * 1024:(g + 1) * 1024]
                                .rearrange("p (c f) -> p c f", f=512),
                            in_=aqv_o[g, 512:768, :]
                                .rearrange("(c p) f -> p c f", p=128))
                    # q/k: raw lands directly in qk_sb; RMS norm + rotary in
                    # place. m-major with head-0's k then q first so attention
                    # h0 starts while h1 is still normalizing. Square on DVE;
                    # rsqrt = exp(-0.5*ln(ms+eps)) keeps ACT in one table set.
                    for n in range(NTT):
                        nc.sync.dma_start(
                            out=qk_sb[:].rearrange("p (m r) -> p m r", m=4)
                                [:, :, n * 512:(n + 1) * 512],
                            in_=aqv_o[n, 0:512, :]
                                .rearrange("(c p) f -> p c f", p=128))
                    for m in (2, 0, 3, 1):
                        for n in range(NTT):
                            raw = qk_sb[:, m * NT + n * 512: m * NT + (n + 1) * 512]
                            sq = st.tile([128, 512], MM, tag="sq")
                            nc.vector.tensor_mul(sq[:], raw, raw)
                            ssq = ps3.tile([128, 512], f32, tag="ops")
                            nc.tensor.matmul(ssq[:], ones_sb[:], sq[:], start=True, stop=True)
                            lt = st.tile([128, 512], f32, tag="lt")
                            nc.scalar.activation(lt[:], ssq[:], Ln, bias=eps_sb[:], scale=1.0 / HD)
                            r = st.tile([128, 512], f32, tag="r")
                            nc.scalar.activation(r[:], lt[:], Exp, scale=-0.5)
                            qn = st.tile([128, 512], MM, tag="qn")
                            nc.vector.tensor_mul(qn[:], raw, r[:])
                            tsw = st.tile([128, 512], MM, tag="tsw")
                            ctile = cs_sb[:, n * 512:(n + 1) * 512]
                            stile = cs_sb[:, NT + n * 512: NT + (n + 1) * 512]
                            nc.vector.tensor_mul(tsw[0:64, :], qn[64:128, :], stile[64:128, :])
                            nc.vector.tensor_mul(tsw[64:128, :], qn[0:64, :], stile[0:64, :])
                            dst = qk_sb[:, m * NT + n * 512: m * NT + (n + 1) * 512]
                            nc.vector.tensor_mul(dst, qn[:], ctile)
                            nc.vector.tensor_add(dst, dst, tsw[:])

                    # prefetch Wo during attention: one 2MB DMA per block
                    wo_blocks = []
                    for on in range(4):
                        wo_sb = wop.tile([128, DC * 512], MM, tag="wo")
                        nc.sync.dma_start(
                            out=wo_sb[:].rearrange("p (c f) -> p c f", f=512),
                            in_=wo_d[:, on * 512:(on + 1) * 512]
                                .rearrange("(c p) f -> p c f", p=128))
                        wo_blocks.append(wo_sb)

                    for h in range(HPC):
                        qoff = h * NT
                        koff = (2 + h) * NT
                        for b in range(B):
                            for qj in range(4):
                                yps = psy.tile([128, 512], f32, tag="yps")
                                dps = psd.tile([128, 512], f32, tag="dps")
                                nkt = 4 * qj + 4
                                qbase = qoff + b * T + qj * 512
                                for kb in range(nkt):
                                    # diagonal blocks: only q-cols >= 128*m live
                                    lo = max(0, (kb - 4 * qj) * 128)
                                    sps = pss.tile([128, 512], f32, tag="sps")
                                    nc.tensor.matmul(
                                        sps[:, lo:],
                                        qk_sb[:, koff + b * T + kb * 128: koff + b * T + (kb + 1) * 128],
                                        qk_sb[:, qbase + lo: qbase + 512],
                                        start=True, stop=True)
                                    e = p2.tile([128, 512], MM, tag="e")
                                    nc.scalar.activation(e[:, lo:], sps[:, lo:], Exp, scale=scale)
                                    if kb >= 4 * qj:
                                        nc.vector.tensor_mul(
                                            e[:, lo:], e[:, lo:], m0_sb[:, 0:512 - lo])
                                    nc.tensor.matmul(dps[:, lo:], ones_sb[:], e[:, lo:],
                                                     start=(kb == 0), stop=(kb == nkt - 1))
                                    tcg = b * KT_PER_B + kb
                                    nc.tensor.matmul(
                                        yps[:, lo:],
                                        v_sb[:, tcg * 256 + h * 128: tcg * 256 + (h + 1) * 128],
                                        e[:, lo:],
                                        start=(kb == 0), stop=(kb == nkt - 1))
                                rcp = p2b.tile([128, 512], f32, tag="rcp")
                                nc.vector.reciprocal(rcp[:], dps[:])
                                yn = p2b.tile([128, 512], MM, tag="yn")
                                nc.vector.tensor_mul(yn[:], yps[:], rcp[:])
                                s = b * 4 + qj
                                nc.sync.dma_start(
                                    out=ay_i[s, h * HD:(h + 1) * HD, :], in_=yn[:])
                    nc.gpsimd.collective_compute(
                        "AllToAll", mybir.AluOpType.bypass,
                        replica_groups=rg,
                        ins=[ay_i.opt()], outs=[ay_o.opt()])

                    # o-proj: y-dim chunk g = 2j + r lives in ay_o[j][r];
                    # one DMA per slot j
                    yt = p3.tile([128, DC * 512], MM, tag="yt")
                    for j in range(N_CORES):
                        nc.sync.dma_start(
                            out=yt[:, j * 1024:(j + 1) * 1024]
                                .rearrange("p (c f) -> p c f", f=512),
                            in_=ay_o[j, :, :].rearrange("(c p) f -> p c f", p=128))
                    for mc in range(4):
                        obig = obp.tile([128, 4 * 512], MM, tag="obig")
                        for on in range(4):
                            ps = ps3.tile([128, 512], f32, tag="ops")
                            for g in range(DC):
                                nc.tensor.matmul(
                                    ps[:],
                                    yt[:, g * 512 + mc * 128: g * 512 + (mc + 1) * 128],
                                    wo_blocks[on][:, g * 512:(g + 1) * 512],
                                    start=(g == 0), stop=(g == DC - 1))
                            nc.vector.tensor_copy(obig[:, on * 512:(on + 1) * 512], ps[:])
                        nc.sync.dma_start(
                            out=y_d[mc * 128:(mc + 1) * 128, :], in_=obig[:])

    nc.compile()
    return nc


def _consts(W, cos, sin):
    import concourse.mybir as mybir
    bf = mybir.dt.np(mybir.dt.bfloat16)
    woT = np.ascontiguousarray(W[3].T.astype(np.float32)).astype(bf)
    wqkT = np.ascontiguousarray(
        np.concatenate([W[0], W[1]], 0).T.astype(np.float32)).astype(bf)
    wvT = np.ascontiguousarray(W[2].T.astype(np.float32)).astype(bf)
    cT = cos.T.astype(np.float32)
    sT = sin.T.astype(np.float32)
    C128 = np.tile(np.concatenate([cT, cT], 0), (1, B)).astype(bf)
    S128 = np.tile(np.concatenate([-sT, sT], 0), (1, B)).astype(bf)
    cs_full = np.ascontiguousarray(np.stack([C128, S128])).reshape(2 * 128, NT)
    m0 = (np.arange(128)[:, None] <= np.arange(512)[None, :]).astype(bf)
    return woT, wqkT, wvT, cs_full, m0


def _prep_inputs(x, W, cos, sin):
    import concourse.mybir as mybir
    bf = mybir.dt.np(mybir.dt.bfloat16)

    xT = np.ascontiguousarray(x.reshape(NT, D).T).astype(bf)
    in_maps = []
    for c in range(N_CORES):
        xs = np.ascontiguousarray(xT[:, c * TS:(c + 1) * TS])
        in_maps.append({"xs": xs})
    return in_maps


def kernel(x, W, cos, sin, scale):
    from concourse.bass_utils import run_bass_kernel_spmd

    x = np.asarray(x, dtype=np.float32)
    W = np.asarray(W, dtype=np.float32)
    cos = np.asarray(cos, dtype=np.float32)
    sin = np.asarray(sin, dtype=np.float32)
    sc = float(np.asarray(scale))

    key = (sc, hash(W.tobytes()), hash(cos.tobytes()), hash(sin.tobytes()))
    if key not in _CACHE:
        woT, wqkT, wvT, cs_full, m0 = _consts(W, cos, sin)
        _CACHE[key] = _build(sc, woT, wqkT, wvT, cs_full, m0)
    nc = _CACHE[key]

    in_maps = _prep_inputs(x, W, cos, sin)
    out = run_bass_kernel_spmd(nc, in_maps, core_ids=list(range(N_CORES)))
    y = np.concatenate([out.results[c]["y"] for c in range(N_CORES)], axis=0)
    return y.astype(np.float32).reshape(B, T, D)


# revision 4
# speedup vs baseline: 1.4675x; 1.4675x over previous
"""Causal self-attention (B=2, T=2048, D=2048, H=16, HD=128) on 8 TRN2 cores.

Tensor-parallel over heads (2 heads/core), token-parallel QKV projection,
with the collectives merged into exactly two AllToAlls (one 6MB qkv+v,
one 2MB y) - collective dispatch/serialization is a dominant per-iteration
cost in this runtime.

Engine balance: every ACT op (Square / Ln / Exp / none else) lives in the
single `natural_log_exp_and_others` table set - rsqrt for RMS-norm is
computed as exp(-0.5*ln(ms+eps)), so the strict-FIFO scalar engine never
pays the ~2.7us ACT table reload, regardless of scheduling order.
PSUM->SBUF copies run on the vector engine (ACT is ~9x slower at copies).
DMAs are batched via rearranged access patterns (v/raw/yt reads one DMA
per A2A slot, output one DMA per row-block).

  - x ships as a per-core 512-token slice of xT (2MB bf16),
  - weights/cos/sin/mask are baked into the NEFF as Const tensors,
  - output returns bf16 (cast to f32 on host).
Matmul layouts: contractions on the partition dim; scores transposed
(S^T = k-chunk @ q^T); softmax denominator via all-ones matmul;
normalization folds into the PSUM->SBUF copy. Matmuls bf16, stats fp32.
"""

import numpy as np

B, T, D = 2, 2048, 2048
H, HD = 16, 128
N_CORES = 8
HPC = H // N_CORES          # heads per core
NT = B * T                  # 4096 tokens, b-major
TS = NT // N_CORES          # 512-token slice per core
DC = D // 128               # 16 contraction chunks
NTT = NT // 512             # 8 token tiles
KT_PER_B = T // 128         # 16 k-tiles per batch row

_CACHE = {}


def _build(scale: float, woT, wqkT, wvT, cs_full, m0, reps: int = 1):
    import concourse.bacc as bacc
    import concourse.mybir as mybir
    import concourse.tile as tile

    f32 = mybir.dt.float32
    MM = mybir.dt.bfloat16
    EPS = float(np.finfo(np.float32).eps)

    nc = bacc.Bacc("TRN2", target_bir_lowering=False, debug=False,
                   num_devices=N_CORES)

    xs_d = nc.dram_tensor("xs", [D, TS], MM, kind="ExternalInput")
    y_d = nc.dram_tensor("y", [TS, D], MM, kind="ExternalOutput")
    # model-load-time constants: no per-execution staging cost
    wo_d = nc.inline_tensor(woT, name="woc")          # [D, D] = Wo^T
    wqk_c = nc.inline_tensor(wqkT, name="wqkc")       # [D, 2D] = (W0||W1)^T
    wv_c = nc.inline_tensor(wvT, name="wvc")          # [D, D] = W2^T
    cs_c = nc.inline_tensor(cs_full, name="csc")      # [256, NT]
    m0_c = nc.inline_tensor(m0, name="m0c")           # [128, 512] triangle

    Sq = mybir.ActivationFunctionType.Square
    Ln = mybir.ActivationFunctionType.Ln
    Exp = mybir.ActivationFunctionType.Exp

    rg = [list(range(N_CORES))]

    with tile.TileContext(nc) as tc:
        with tc.tile_pool(name="dram", bufs=1, space="DRAM") as dram, \
             tc.tile_pool(name="res", bufs=1) as res:
            # qkv A2As: av carries v token-major quadrants [128 tok x
            # 256 vdim] x (2x2) and fires early (hidden under qk compute);
            # aqk carries q/k dims of core s (q h0|q h1|k h0|k h1),
            # cols = my 512 tokens
            av_i = dram.tile([N_CORES, 256, 512], MM, tag="avi", name="avi")
            av_o = dram.tile([N_CORES, 256, 512], MM, tag="avo", name="avo")
            aqk_i = dram.tile([N_CORES, 512, 512], MM, tag="aqki", name="aqki")
            aqk_o = dram.tile([N_CORES, 512, 512], MM, tag="aqko", name="aqko")
            # merged y A2A: slot s = [my 2 heads x 128 rows, tokens of s]
            ay_i = dram.tile([N_CORES, HPC * HD, TS], MM, tag="ayi", name="ayi")
            ay_o = dram.tile([N_CORES, HPC * HD, TS], MM, tag="ayo", name="ayo")

            qk_sb = res.tile([128, 4 * NT], MM, tag="qk")
            v_sb = res.tile([128, (NT // 128) * (HPC * HD)], MM, tag="v")
            m0_sb = res.tile([128, 512], MM, tag="m0")
            ones_sb = res.tile([128, 128], MM, tag="ones")
            eps_sb = res.tile([128, 1], f32, tag="eps")
            nc.vector.memset(eps_sb[:], EPS)
            nc.vector.memset(ones_sb[:], 1.0)
            nc.sync.dma_start(out=m0_sb[:], in_=m0_c[:, :])

            for _rep in range(reps):
                # ------- Phase 1: token-parallel QKV into one A2A -------
                with tc.tile_pool(name="p1", bufs=1) as p1, \
                     tc.tile_pool(name="wp", bufs=2) as wp, \
                     tc.tile_pool(name="cp", bufs=4) as cp, \
                     tc.tile_pool(name="ps1", bufs=3, space="PSUM") as ps1:
                    xsb = p1.tile([128, DC * 512], MM, tag="xsb")
                    nc.sync.dma_start(
                        out=xsb[:].rearrange("p (c f) -> p c f", f=512),
                        in_=xs_d[:, :].rearrange("(c p) f -> p c f", p=128))

                    # v = x @ Wv -> [my tok, vdim]; dest 2ob gets cols 0:256,
                    # dest 2ob+1 cols 256:512; quadrant (t4//2, t4%2)
                    for ob in range(4):
                        wvb = wp.tile([128, DC * 512], MM, tag="wb", name="wvb")
                        nc.sync.dma_start(
                            out=wvb[:].rearrange("p (c f) -> p c f", f=512),
                            in_=wv_c[:, ob * 512:(ob + 1) * 512]
                                .rearrange("(c p) f -> p c f", p=128))
                        for t4 in range(4):
                            vps = ps1.tile([128, 512], f32, tag="vps")
                            for dc in range(DC):
                                nc.tensor.matmul(
                                    vps[:],
                                    xsb[:, dc * 512 + t4 * 128: dc * 512 + (t4 + 1) * 128],
                                    wvb[:, dc * 512:(dc + 1) * 512],
                                    start=(dc == 0), stop=(dc == DC - 1))
                            vb = cp.tile([128, 512], MM, tag="vb")
                            nc.vector.tensor_copy(vb[:], vps[:])
                            r0 = (t4 // 2) * 128
                            c0 = (t4 % 2) * 256
                            nc.sync.dma_start(
                                out=av_i[2 * ob, r0:r0 + 128, c0:c0 + 256],
                                in_=vb[:, 0:256])
                            nc.sync.dma_start(
                                out=av_i[2 * ob + 1, r0:r0 + 128, c0:c0 + 256],
                                in_=vb[:, 256:512])
                    nc.gpsimd.collective_compute(
                        "AllToAll", mybir.AluOpType.bypass,
                        replica_groups=rg,
                        ins=[av_i.opt()], outs=[av_o.opt()])

                    # q||k = x @ (W0||W1): [out-dim 128, my 512 tok] chunks
                    for ob in range(8):
                        wqb = wp.tile([128, DC * 512], MM, tag="wb", name="wqb")
                        nc.sync.dma_start(
                            out=wqb[:].rearrange("p (c f) -> p c f", f=512),
                            in_=wqk_c[:, ob * 512:(ob + 1) * 512]
                                .rearrange("(c p) f -> p c f", p=128))
                        for oc in range(4):
                            qps = ps1.tile([128, 512], f32, tag="qps")
                            for dc in range(DC):
                                nc.tensor.matmul(
                                    qps[:],
                                    wqb[:, dc * 512 + oc * 128: dc * 512 + (oc + 1) * 128],
                                    xsb[:, dc * 512:(dc + 1) * 512],
                                    start=(dc == 0), stop=(dc == DC - 1))
                            qb = cp.tile([128, 512], MM, tag="qb")
                            nc.vector.tensor_copy(qb[:], qps[:])
                            o = ob * 4 + oc               # global 128-out chunk
                            if o < 16:
                                s, row = o // 2, (o % 2) * 128
                            else:
                                s, row = (o - 16) // 2, 256 + ((o - 16) % 2) * 128
                            nc.sync.dma_start(
                                out=aqk_i[s, row:row + 128, :], in_=qb[:])
                    nc.gpsimd.collective_compute(
                        "AllToAll", mybir.AluOpType.bypass,
                        replica_groups=rg,
                        ins=[aqk_i.opt()], outs=[aqk_o.opt()])

                # ----- Phase 2: norm+rotary, attention, y A2A, o-proj -----
                with tc.tile_pool(name="st", bufs=3) as st, \
                     tc.tile_pool(name="p2", bufs=4) as p2, \
                     tc.tile_pool(name="p2b", bufs=2) as p2b, \
                     tc.tile_pool(name="pss", bufs=2, space="PSUM") as pss, \
                     tc.tile_pool(name="psd", bufs=2, space="PSUM") as psd, \
                     tc.tile_pool(name="psy", bufs=2, space="PSUM") as psy, \
                     tc.tile_pool(name="p3", bufs=1) as p3, \
                     tc.tile_pool(name="wop", bufs=4) as wop, \
                     tc.tile_pool(name="ob", bufs=2) as obp, \
                     tc.tile_pool(name="ps3", bufs=2, space="PSUM") as ps3:
                    cs_sb = p3.tile([128, 2 * NT], MM, tag="cs")
                    nc.sync.dma_start(
                        out=cs_sb[:].rearrange("p (c f) -> p c f", f=NT),
                        in_=cs_c[:, :].rearrange("(c p) f -> p c f", p=128))
                    # v: one DMA per slot; quadrant layout maps exactly onto
                    # v_sb's (tcg*256) column order
                    for g in range(NTT):
                        nc.sync.dma_start(
                            out=v_sb[:, g * 1024:(g + 1) * 1024]
                                .rearrange("p (c f) -> p c f", f=512),
                            in_=av_o[g, 0:256, :]
                                .rearrange("(c p) f -> p c f", p=128))
                    # q/k: raw lands directly in qk_sb; RMS norm + rotary in
                    # place. m-major with head-0's k then q first so attention
                    # h0 starts while h1 is still normalizing. Square on DVE;
                    # rsqrt = exp(-0.5*ln(ms+eps)) keeps ACT in one table set.
                    for n in range(NTT):
                        nc.sync.dma_start(
                            out=qk_sb[:].rearrange("p (m r) -> p m r", m=4)
                                [:, :, n * 512:(n + 1) * 512],
                            in_=aqk_o[n, 0:512, :]
                                .rearrange("(c p) f -> p c f", p=128))
                    for m in (2, 0, 3, 1):
                        for n in range(NTT):
                            raw = qk_sb[:, m * NT + n * 512: m * NT + (n + 1) * 512]
                            sq = st.tile([128, 512], MM, tag="sq")
                            nc.vector.tensor_mul(sq[:], raw, raw)
                            ssq = ps3.tile([128, 512], f32, tag="ops")
                            nc.tensor.matmul(ssq[:], ones_sb[:], sq[:], start=True, stop=True)
                            lt = st.tile([128, 512], f32, tag="lt")
                            nc.scalar.activation(lt[:], ssq[:], Ln, bias=eps_sb[:], scale=1.0 / HD)
                            r = st.tile([128, 512], f32, tag="r")
                            nc.scalar.activation(r[:], lt[:], Exp, scale=-0.5)
                            qn = st.tile([128, 512], MM, tag="qn")
                            nc.vector.tensor_mul(qn[:], raw, r[:])
                            tsw = st.tile([128, 512], MM, tag="tsw")
                            ctile = cs_sb[:, n * 512:(n + 1) * 512]
                            stile = cs_sb[:, NT + n * 512: NT + (n + 1) * 512]
                            nc.vector.tensor_mul(tsw[0:64, :], qn[64:128, :], stile[64:128, :])
                            nc.vector.tensor_mul(tsw[64:128, :], qn[0:64, :], stile[0:64, :])
                            dst = qk_sb[:, m * NT + n * 512: m * NT + (n + 1) * 512]
                            nc.vector.tensor_mul(dst, qn[:], ctile)
                            nc.vector.tensor_add(dst, dst, tsw[:])

                    # prefetch Wo during attention: one 2MB DMA per block
                    wo_blocks = []
                    for on in range(4):
                        wo_sb = wop.tile([128, DC * 512], MM, tag="wo")
                        nc.sync.dma_start(
                            out=wo_sb[:].rearrange("p (c f) -> p c f", f=512),
                            in_=wo_d[:, on * 512:(on + 1) * 512]
                                .rearrange("(c p) f -> p c f", p=128))
                        wo_blocks.append(wo_sb)

                    for h in range(HPC):
                        qoff = h * NT
                        koff = (2 + h) * NT
                        for b in range(B):
                            for qj in range(4):
                                yps = psy.tile([128, 512], f32, tag="yps")
                                dps = psd.tile([128, 512], f32, tag="dps")
                                nkt = 4 * qj + 4
                                qbase = qoff + b * T + qj * 512
                                for kb in range(nkt):
                                    # diagonal blocks: only q-cols >= 128*m live
                                    lo = max(0, (kb - 4 * qj) * 128)
                                    sps = pss.tile([128, 512], f32, tag="sps")
                                    nc.tensor.matmul(
                                        sps[:, lo:],
                                        qk_sb[:, koff + b * T + kb * 128: koff + b * T + (kb + 1) * 128],
                                        qk_sb[:, qbase + lo: qbase + 512],
                                        start=True, stop=True)
                                    e = p2.tile([128, 512], MM, tag="e")
                                    nc.scalar.activation(e[:, lo:], sps[:, lo:], Exp, scale=scale)
                                    if kb >= 4 * qj:
                                        nc.vector.tensor_mul(
                                            e[:, lo:], e[:, lo:], m0_sb[:, 0:512 - lo])
                                    nc.tensor.matmul(dps[:, lo:], ones_sb[:], e[:, lo:],
                                                     start=(kb == 0), stop=(kb == nkt - 1))
                                    tcg = b * KT_PER_B + kb
                                    nc.tensor.matmul(
                                        yps[:, lo:],
                                        v_sb[:, tcg * 256 + h * 128: tcg * 256 + (h + 1) * 128],
                                        e[:, lo:],
                                        start=(kb == 0), stop=(kb == nkt - 1))
                                rcp = p2b.tile([128, 512], f32, tag="rcp")
                                nc.vector.reciprocal(rcp[:], dps[:])
                                yn = p2b.tile([128, 512], MM, tag="yn")
                                nc.vector.tensor_mul(yn[:], yps[:], rcp[:])
                                s = b * 4 + qj
                                nc.sync.dma_start(
                                    out=ay_i[s, h * HD:(h + 1) * HD, :], in_=yn[:])
                    nc.gpsimd.collective_compute(
                        "AllToAll", mybir.AluOpType.bypass,
                        replica_groups=rg,
                        ins=[ay_i.opt()], outs=[ay_o.opt()])

                    # o-proj: y-dim chunk g = 2j + r lives in ay_o[j][r];
                    # one DMA per slot j
                    yt = p3.tile([128, DC * 512], MM, tag="yt")
                    for j in range(N_CORES):
                        nc.sync.dma_start(
                            out=yt[:, j * 1024:(j + 1) * 1024]
                                .rearrange("p (c f) -> p c f", f=512),
                            in_=ay_o[j, :, :].rearrange("(c p) f -> p c f", p=128))
                    for mc in range(4):
                        obig = obp.tile([128, 4 * 512], MM, tag="obig")
                        for on in range(4):
                            ps = ps3.tile([128, 512], f32, tag="ops")
                            for g in range(DC):
                                nc.tensor.matmul(
                                    ps[:],
                                    yt[:, g * 512 + mc * 128: g * 512 + (mc + 1) * 128],
                                    wo_blocks[on][:, g * 512:(g + 1) * 512],
                                    start=(g == 0), stop=(g == DC - 1))
                            nc.vector.tensor_copy(obig[:, on * 512:(on + 1) * 512], ps[:])
                        nc.sync.dma_start(
                            out=y_d[mc * 128:(mc + 1) * 128, :], in_=obig[:])

    nc.compile()
    return nc


def _consts(W, cos, sin):
    import concourse.mybir as mybir
    bf = mybir.dt.np(mybir.dt.bfloat16)
    woT = np.ascontiguousarray(W[3].T.astype(np.float32)).astype(bf)
    wqkT = np.ascontiguousarray(
        np.concatenate([W[0], W[1]], 0).T.astype(np.float32)).astype(bf)
    wvT = np.ascontiguousarray(W[2].T.astype(np.float32)).astype(bf)
    cT = cos.T.astype(np.float32)
    sT = sin.T.astype(np.float32)
    C128 = np.tile(np.concatenate([cT, cT], 0), (1, B)).astype(bf)
    S128 = np.tile(np.concatenate([-sT, sT], 0), (1, B)).astype(bf)
    cs_full = np.ascontiguousarray(np.stack([C128, S128])).reshape(2 * 128, NT)
    m0 = (np.arange(128)[:, None] <= np.arange(512)[None, :]).astype(bf)
    return woT, wqkT, wvT, cs_full, m0


def _prep_inputs(x, W, cos, sin):
    import concourse.mybir as mybir
    bf = mybir.dt.np(mybir.dt.bfloat16)

    xT = np.ascontiguousarray(x.reshape(NT, D).T).astype(bf)
    in_maps = []
    for c in range(N_CORES):
        xs = np.ascontiguousarray(xT[:, c * TS:(c + 1) * TS])
        in_maps.append({"xs": xs})
    return in_maps


def kernel(x, W, cos, sin, scale):
    from concourse.bass_utils import run_bass_kernel_spmd

    x = np.asarray(x, dtype=np.float32)
    W = np.asarray(W, dtype=np.float32)
    cos = np.asarray(cos, dtype=np.float32)
    sin = np.asarray(sin, dtype=np.float32)
    sc = float(np.asarray(scale))

    key = (sc, hash(W.tobytes()), hash(cos.tobytes()), hash(sin.tobytes()))
    if key not in _CACHE:
        woT, wqkT, wvT, cs_full, m0 = _consts(W, cos, sin)
        _CACHE[key] = _build(sc, woT, wqkT, wvT, cs_full, m0)
    nc = _CACHE[key]

    in_maps = _prep_inputs(x, W, cos, sin)
    out = run_bass_kernel_spmd(nc, in_maps, core_ids=list(range(N_CORES)))
    y = np.concatenate([out.results[c]["y"] for c in range(N_CORES)], axis=0)
    return y.astype(np.float32).reshape(B, T, D)
